# revision 15
# baseline (speedup 1.0000x reference)
"""MinGRU Trainium2 kernel (fp8 DoubleRow edition).

Problem: x (8, 4096, 1024) fp32; Wz, Wh (1024, 1024); bz, bh (1024,).
    k = x @ Wz.T + bz ; z = sigmoid(k)
    p = x @ Wh.T + bh ; g = where(p >= 0, p + 0.5, sigmoid(p))
    h_t = (1 - z_t) * h_{t-1} + z_t * g_t   (h_0 = 0.5)

Sharding: data-parallel over batch, one batch element per NeuronCore (8 cores).

Per-core design:
  * GEMMs run in fp8e4 (e4m3) with MatmulPerfMode.DoubleRow: 2 k-tiles of 128
    packed per pass, 0.5 cycles/row -> ~2x the fp32r matmul rate.  x is scaled
    by 32, W by 4096 on the host (both within e4m3 range); the 1/(32*4096)
    descale is folded into the activations' scale operand.  Measured end2end
    rel err of the quantized pipeline on the fixed harness inputs: 1.33e-2
    (gate 2e-2).
  * Everything transposed: H on partitions, S on the free axis; 1024-wide
    strips, PSUM tiles [128,1024] spanning 2 banks (matmuls write 512-wide
    bank halves), 2 tags x 2 bufs = all 8 banks.
  * Post-GEMM algebra balanced across engines (a = 1-z = sigmoid(-k),
    g = max(sigmoid(p'), p'+0.5) which is exactly the reference g):
      ScalarE: a = sig(-k/S - bz), sp = sig(p/S + bh), and (m<4) pb = p/S+bh+.5
      DVE:     (m 4,5) pb, g = max(sp, pb), h = scan(a, bb, mult, subtract)
      GpSimd:  (m 6,7) pb, bb = (a-1)*g
    scan: state = a*state - bb = a*state + (1-a)*g, fp32 state, bf16 out.
  * All intermediates bf16 (DVE 2x/4x modes); h stored/DMA'd as bf16.
"""

import os
import sys

import numpy as np

for _p in ("/opt/trn_rl_repo", "/root/.axon_site/_ro/trn_rl_repo"):
    if os.path.isdir(_p) and _p not in sys.path:
        sys.path.insert(0, _p)

import concourse.bass as bass  # noqa: E402
import concourse.mybir as mybir  # noqa: E402
import concourse.tile as tile  # noqa: E402
from concourse import bacc  # noqa: E402
from concourse.bass_utils import run_bass_kernel_spmd  # noqa: E402

F32 = mybir.dt.float32
F32R = mybir.dt.float32r
F8 = mybir.dt.float8e4
BF16 = mybir.dt.bfloat16
N_CORES = 8
B, S, D, H = 8, 4096, 1024, 1024
NM = H // 128  # m tiles (output partition blocks)
NP = D // 256  # DoubleRow k pairs (256 contraction rows each)
SX = 32.0  # x fp8 scale
SW = 4096.0  # weight fp8 scale
ISCALE = 1.0 / (SX * SW)

_cache: dict = {}


def make_units(seq_len: int):
    """1024-wide strips, with the final 1024 split 512/256/128/128 so the
    end-of-kernel pipeline drain runs on narrow tiles."""
    units = []
    t = 0
    while seq_len - t > 1024:
        units.append((t, 1024))
        t += 1024
    r = seq_len - t
    tail = []
    while r > 256:
        tail.append(r // 2)
        r -= r // 2
    if r >= 128:
        tail += [r // 2, r - r // 2]
    else:
        tail.append(r)
    for w in tail:
        units.append((t, w))
        t += w
    return units


def build_nc(seq_len: int = S, n_cores: int = N_CORES):
    nc = bacc.Bacc(
        "TRN2", target_bir_lowering=False, debug=False, num_devices=n_cores
    )

    xT_d = nc.dram_tensor("xT8", [D, seq_len], F8, kind="ExternalInput")
    wz_d = nc.dram_tensor("wz8", [D, H], F8, kind="ExternalInput")
    wh_d = nc.dram_tensor("wh8", [D, H], F8, kind="ExternalInput")
    nbz_d = nc.dram_tensor("nbz", [H], F32, kind="ExternalInput")
    bh_d = nc.dram_tensor("bh", [H], F32, kind="ExternalInput")
    bh05_d = nc.dram_tensor("bh05", [H], F32, kind="ExternalInput")
    hT_d = nc.dram_tensor("hT", [H, seq_len], BF16, kind="ExternalOutput")

    AF = mybir.ActivationFunctionType
    OP = mybir.AluOpType
    DR = mybir.MatmulPerfMode.DoubleRow

    units = make_units(seq_len)

    with tile.TileContext(nc) as tc:
        with (
            tc.tile_pool(name="singles", bufs=1) as singles,
            tc.tile_pool(name="xs", bufs=3) as xpool,
            tc.tile_pool(name="work", bufs=8) as work,
            tc.tile_pool(name="hbuf", bufs=2) as hpool,
            tc.tile_pool(name="psum", bufs=2, space="PSUM") as psum,
        ):
            # PE warm-up while the first DMAs land (clock ramp to 2.4 GHz).
            warm = singles.tile([128, 512], F32, tag="warm")
            nc.gpsimd.memset(warm[:], 0.0)
            wps = psum.tile([128, 1024], F32, tag="kp")
            for i in range(12):
                nc.tensor.matmul(
                    wps[:, :512], lhsT=warm[:, :128].bitcast(F32R),
                    rhs=warm[:].bitcast(F32R),
                    start=(i == 0), stop=(i == 11),
                )
            # Biases first (tiny, gate every activation).
            nbz_sb = singles.tile([128, NM], F32, tag="nbz")
            nc.sync.dma_start(
                out=nbz_sb, in_=nbz_d.ap().rearrange("(m p) -> p m", p=128)
            )
            bh_sb = singles.tile([128, NM], F32, tag="bh")
            nc.sync.dma_start(
                out=bh_sb, in_=bh_d.ap().rearrange("(m p) -> p m", p=128)
            )
            bh05_sb = singles.tile([128, NM], F32, tag="bh05")
            nc.sync.dma_start(
                out=bh05_sb, in_=bh05_d.ap().rearrange("(m p) -> p m", p=128)
            )
            # First strip of x + the m<4 weight halves, interleaved per pair so
            # the first matmul group unblocks asap.
            tw0 = units[0][1]
            xs0 = [None] * NP
            wz_sb = [[None, None] for _ in range(NP)]
            wh_sb = [[None, None] for _ in range(NP)]
            for j in range(NP):
                xt = xpool.tile([128, 2, 1024], F8, tag=f"xs{j}")
                nc.sync.dma_start(
                    out=xt[:, :, :tw0],
                    in_=xT_d.ap()[256 * j:256 * (j + 1), 0:tw0].rearrange(
                        "(two p) s -> p two s", two=2
                    ),
                )
                xs0[j] = xt
                wz = singles.tile([128, 2, H // 2], F8, tag=f"wz{j}_0")
                nc.sync.dma_start(
                    out=wz,
                    in_=wz_d.ap()[256 * j:256 * (j + 1), 0:H // 2].rearrange(
                        "(two p) h -> p two h", two=2
                    ),
                )
                wz_sb[j][0] = wz
                wh = singles.tile([128, 2, H // 2], F8, tag=f"wh{j}_0")
                nc.sync.dma_start(
                    out=wh,
                    in_=wh_d.ap()[256 * j:256 * (j + 1), 0:H // 2].rearrange(
                        "(two p) h -> p two h", two=2
                    ),
                )
                wh_sb[j][0] = wh
            for j in range(NP):
                wz = singles.tile([128, 2, H // 2], F8, tag=f"wz{j}_1")
                nc.sync.dma_start(
                    out=wz,
                    in_=wz_d.ap()[256 * j:256 * (j + 1), H // 2:H].rearrange(
                        "(two p) h -> p two h", two=2
                    ),
                )
                wz_sb[j][1] = wz
                wh = singles.tile([128, 2, H // 2], F8, tag=f"wh{j}_1")
                nc.sync.dma_start(
                    out=wh,
                    in_=wh_d.ap()[256 * j:256 * (j + 1), H // 2:H].rearrange(
                        "(two p) h -> p two h", two=2
                    ),
                )
                wh_sb[j][1] = wh

            h_prev: list = [None] * NM

            # Global software pipeline over steps s = (unit, m).  Stages are
            # issued with explicit delays so every in-order engine queue only
            # ever sees instructions whose inputs are at least one step old:
            #   step s:  matmuls(s), a-act(s), z(s)=1-a on GpSimd,
            #            sp/pb-acts(s-1), g(s-1) on DVE, bb(s-1)=z*g on GpSimd,
            #            scan(s-2) + h-DMA(s-2) on DVE/SP.
            # This keeps Scalar from embedding waits on in-flight PE groups
            # (a-act is the only act coupled to a fresh GEMM) and gives bb a
            # full step of slack before the scan consumes it.
            stage1q: list = []  # (m, pp, a, z, tw, ts_sl)
            stage2q: list = []  # (m, a, bb, tw, ts_sl)

            def stage2(keep):
                while len(stage2q) > keep:
                    m2, a2, bb2, tw2, sl2 = stage2q.pop(0)
                    h = hpool.tile([128, 1024], BF16, tag=f"h{m2}")
                    if h_prev[m2] is None:
                        init = 0.5
                    else:
                        pt, pw = h_prev[m2]
                        init = pt[:, pw - 1:pw]
                    nc.vector.tensor_tensor_scan(
                        out=h[:, :tw2], data0=a2[:, :tw2], data1=bb2[:, :tw2],
                        initial=init, op0=OP.mult, op1=OP.add,
                    )
                    h_prev[m2] = (h, tw2)
                    nc.sync.dma_start(
                        out=hT_d.ap()[m2 * 128:(m2 + 1) * 128, sl2],
                        in_=h[:, :tw2],
                    )

            def stage1(keep):
                while len(stage1q) > keep:
                    m1, pp1, a1, z1, tw1, sl1 = stage1q.pop(0)
                    sp = work.tile([128, 1024], BF16, tag="sp")
                    nc.scalar.activation(
                        out=sp[:, :tw1], in_=pp1[:, :tw1], func=AF.Sigmoid,
                        bias=bh_sb[:, m1:m1 + 1], scale=ISCALE,
                    )
                    pb = work.tile([128, 1024], BF16, tag="pb")
                    nc.scalar.activation(
                        out=pb[:, :tw1], in_=pp1[:, :tw1], func=AF.Identity,
                        bias=bh05_sb[:, m1:m1 + 1], scale=ISCALE,
                    )
                    # g = max(sigmoid(p'), p' + 0.5)  == reference g(p')
                    g = work.tile([128, 1024], BF16, tag="g")
                    nc.vector.tensor_tensor(
                        out=g[:, :tw1], in0=sp[:, :tw1], in1=pb[:, :tw1],
                        op=OP.max,
                    )
                    bb = work.tile([128, 1024], BF16, tag="bb")
                    nc.gpsimd.tensor_tensor(
                        out=bb[:, :tw1], in0=z1[:, :tw1], in1=g[:, :tw1],
                        op=OP.mult,
                    )
                    stage2q.append((m1, a1, bb, tw1, sl1))
                    stage2(1)

            for u, (ts0, tw) in enumerate(units):
                ts_sl = slice(ts0, ts0 + tw)
                if u == 0:
                    xs = xs0
                else:
                    xs = []
                    for j in range(NP):
                        xt = xpool.tile([128, 2, 1024], F8, tag=f"xs{j}")
                        nc.sync.dma_start(
                            out=xt[:, :, :tw],
                            in_=xT_d.ap()[
                                256 * j:256 * (j + 1), ts_sl
                            ].rearrange("(two p) s -> p two s", two=2),
                        )
                        xs.append(xt)
                for m in range(NM):
                    hf, ml = divmod(m, NM // 2)
                    m_sl = slice(ml * 128, (ml + 1) * 128)
                    kp = psum.tile([128, 1024], F32, tag="kp")
                    pp = psum.tile([128, 1024], F32, tag="pp")
                    for dst, w_sb in ((kp, wz_sb), (pp, wh_sb)):
                        for c0 in range(0, tw, 512):
                            cw = min(512, tw - c0)
                            for j in range(NP):
                                nc.tensor.matmul(
                                    dst[:, c0:c0 + cw],
                                    lhsT=w_sb[j][hf][:, :, m_sl],
                                    rhs=xs[j][:, :, c0:c0 + cw],
                                    start=(j == 0),
                                    stop=(j == NP - 1),
                                    perf_mode=DR,
                                )
                    # a = sigmoid(-(k/S + bz)) = 1 - z
                    a = work.tile([128, 1024], BF16, tag="a")
                    nc.scalar.activation(
                        out=a[:, :tw], in_=kp[:, :tw], func=AF.Sigmoid,
                        bias=nbz_sb[:, m:m + 1], scale=-ISCALE,
                    )
                    # z = 1 - a on GpSimd (off the Scalar/DVE critical paths)
                    z = work.tile([128, 1024], BF16, tag="z")
                    nc.gpsimd.tensor_scalar(
                        out=z[:, :tw], in0=a[:, :tw], scalar1=-1.0,
                        scalar2=1.0, op0=OP.mult, op1=OP.add,
                    )
                    stage1q.append((m, pp, a, z, tw, ts_sl))
                    stage1(1)

            stage1(0)
            stage2(0)

    nc.compile()
    return nc


def _prep_inputs(x, Wz, bz, Wh, bh):
    import ml_dtypes

    E4 = ml_dtypes.float8_e4m3
    x = np.asarray(x, np.float32)
    xq = (x * SX).astype(E4)
    wz8 = np.ascontiguousarray((np.asarray(Wz, np.float32).T * SW).astype(E4))
    wh8 = np.ascontiguousarray((np.asarray(Wh, np.float32).T * SW).astype(E4))
    nbz = np.ascontiguousarray(-np.asarray(bz, np.float32))
    bh32 = np.ascontiguousarray(np.asarray(bh, np.float32))
    bh05 = np.ascontiguousarray(bh32 + np.float32(0.5))
    return [
        {
            "xT8": np.ascontiguousarray(xq[b].T),
            "wz8": wz8,
            "wh8": wh8,
            "nbz": nbz,
            "bh": bh32,
            "bh05": bh05,
        }
        for b in range(x.shape[0])
    ]


def kernel(x, Wz, bz, Wh, bh):
    if "nc" not in _cache:
        _cache["nc"] = build_nc()
    nc = _cache["nc"]
    in_maps = _prep_inputs(x, Wz, bz, Wh, bh)
    res = run_bass_kernel_spmd(nc, in_maps, list(range(N_CORES)))
    out = np.empty((B, S, H), np.float32)
    for b in range(N_CORES):
        out[b] = res.results[b]["hT"].astype(np.float32).T
    return out


# revision 17
# speedup vs baseline: 1.2422x; 1.2422x over previous
"""MinGRU Trainium2 kernel (fp8 DoubleRow edition).

Problem: x (8, 4096, 1024) fp32; Wz, Wh (1024, 1024); bz, bh (1024,).
    k = x @ Wz.T + bz ; z = sigmoid(k)
    p = x @ Wh.T + bh ; g = where(p >= 0, p + 0.5, sigmoid(p))
    h_t = (1 - z_t) * h_{t-1} + z_t * g_t   (h_0 = 0.5)

Sharding: data-parallel over batch, one batch element per NeuronCore (8 cores).

Per-core design:
  * GEMMs run in fp8e4 (e4m3) with MatmulPerfMode.DoubleRow: 2 k-tiles of 128
    packed per pass, 0.5 cycles/row -> ~2x the fp32r matmul rate.  x is scaled
    by 32, W by 4096 on the host (both within e4m3 range); the 1/(32*4096)
    descale is folded into the activations' scale operand.  Measured end2end
    rel err of the quantized pipeline on the fixed harness inputs: 1.33e-2
    (gate 2e-2).
  * Everything transposed: H on partitions, S on the free axis; 1024-wide
    strips, PSUM tiles [128,1024] spanning 2 banks (matmuls write 512-wide
    bank halves), 2 tags x 2 bufs = all 8 banks.
  * Post-GEMM algebra balanced across engines (a = 1-z = sigmoid(-k),
    g = max(sigmoid(p'), p'+0.5) which is exactly the reference g):
      ScalarE: a = sig(-k/S - bz), sp = sig(p/S + bh), and (m<4) pb = p/S+bh+.5
      DVE:     (m 4,5) pb, g = max(sp, pb), h = scan(a, bb, mult, subtract)
      GpSimd:  (m 6,7) pb, bb = (a-1)*g
    scan: state = a*state - bb = a*state + (1-a)*g, fp32 state, bf16 out.
  * All intermediates bf16 (DVE 2x/4x modes); h stored/DMA'd as bf16.
"""

import os
import sys

import numpy as np

for _p in ("/opt/trn_rl_repo", "/root/.axon_site/_ro/trn_rl_repo"):
    if os.path.isdir(_p) and _p not in sys.path:
        sys.path.insert(0, _p)

import concourse.bass as bass  # noqa: E402
import concourse.mybir as mybir  # noqa: E402
import concourse.tile as tile  # noqa: E402
from concourse import bacc  # noqa: E402
from concourse.bass_utils import run_bass_kernel_spmd  # noqa: E402

F32 = mybir.dt.float32
F32R = mybir.dt.float32r
F8 = mybir.dt.float8e4
BF16 = mybir.dt.bfloat16
N_CORES = 8
B, S, D, H = 8, 4096, 1024, 1024
NM = H // 128  # m tiles (output partition blocks)
NP = D // 256  # DoubleRow k pairs (256 contraction rows each)
SX = 32.0  # x fp8 scale
SW = 4096.0  # weight fp8 scale
ISCALE = 1.0 / (SX * SW)

_cache: dict = {}


def make_units(seq_len: int):
    """1024-wide strips, with the final 1024 split 512/256/128/128 so the
    end-of-kernel pipeline drain runs on narrow tiles."""
    units = []
    t = 0
    while seq_len - t > 1024:
        units.append((t, 1024))
        t += 1024
    r = seq_len - t
    tail = []
    while r > 256:
        tail.append(r // 2)
        r -= r // 2
    if r >= 128:
        tail += [r // 2, r - r // 2]
    else:
        tail.append(r)
    for w in tail:
        units.append((t, w))
        t += w
    return units


def build_nc(seq_len: int = S, n_cores: int = N_CORES):
    nc = bacc.Bacc(
        "TRN2", target_bir_lowering=False, debug=False, num_devices=n_cores
    )

    xT_d = nc.dram_tensor("xT8", [D, seq_len], F8, kind="ExternalInput")
    wz_d = nc.dram_tensor("wz8", [D, H], F8, kind="ExternalInput")
    wh_d = nc.dram_tensor("wh8", [D, H], F8, kind="ExternalInput")
    nbz_d = nc.dram_tensor("nbz", [H], F32, kind="ExternalInput")
    bh_d = nc.dram_tensor("bh", [H], F32, kind="ExternalInput")
    bh05_d = nc.dram_tensor("bh05", [H], F32, kind="ExternalInput")
    hT_d = nc.dram_tensor("hT", [H, seq_len], BF16, kind="ExternalOutput")

    AF = mybir.ActivationFunctionType
    OP = mybir.AluOpType
    DR = mybir.MatmulPerfMode.DoubleRow

    units = make_units(seq_len)

    with tile.TileContext(nc) as tc:
        with (
            tc.tile_pool(name="singles", bufs=1) as singles,
            tc.tile_pool(name="xs", bufs=3) as xpool,
            tc.tile_pool(name="work", bufs=8) as work,
            tc.tile_pool(name="hbuf", bufs=2) as hpool,
            tc.tile_pool(name="psum", bufs=2, space="PSUM") as psum,
        ):
            # PE warm-up while the first DMAs land (clock ramp to 2.4 GHz).
            warm = singles.tile([128, 512], F32, tag="warm")
            nc.gpsimd.memset(warm[:], 0.0)
            wps = psum.tile([128, 1024], F32, tag="kp")
            for i in range(12):
                nc.tensor.matmul(
                    wps[:, :512], lhsT=warm[:, :128].bitcast(F32R),
                    rhs=warm[:].bitcast(F32R),
                    start=(i == 0), stop=(i == 11),
                )
            # Biases first (tiny, gate every activation).
            nbz_sb = singles.tile([128, NM], F32, tag="nbz")
            nc.sync.dma_start(
                out=nbz_sb, in_=nbz_d.ap().rearrange("(m p) -> p m", p=128)
            )
            bh_sb = singles.tile([128, NM], F32, tag="bh")
            nc.sync.dma_start(
                out=bh_sb, in_=bh_d.ap().rearrange("(m p) -> p m", p=128)
            )
            bh05_sb = singles.tile([128, NM], F32, tag="bh05")
            nc.sync.dma_start(
                out=bh05_sb, in_=bh05_d.ap().rearrange("(m p) -> p m", p=128)
            )
            # First strip of x + the m<4 weight halves, interleaved per pair so
            # the first matmul group unblocks asap.
            tw0 = units[0][1]
            xs0 = [None] * NP
            wz_sb = [[None, None] for _ in range(NP)]
            wh_sb = [[None, None] for _ in range(NP)]
            for j in range(NP):
                xt = xpool.tile([128, 2, 1024], F8, tag=f"xs{j}")
                nc.sync.dma_start(
                    out=xt[:, :, :tw0],
                    in_=xT_d.ap()[256 * j:256 * (j + 1), 0:tw0].rearrange(
                        "(two p) s -> p two s", two=2
                    ),
                )
                xs0[j] = xt
                wz = singles.tile([128, 2, H // 2], F8, tag=f"wz{j}_0")
                nc.sync.dma_start(
                    out=wz,
                    in_=wz_d.ap()[256 * j:256 * (j + 1), 0:H // 2].rearrange(
                        "(two p) h -> p two h", two=2
                    ),
                )
                wz_sb[j][0] = wz
                wh = singles.tile([128, 2, H // 2], F8, tag=f"wh{j}_0")
                nc.sync.dma_start(
                    out=wh,
                    in_=wh_d.ap()[256 * j:256 * (j + 1), 0:H // 2].rearrange(
                        "(two p) h -> p two h", two=2
                    ),
                )
                wh_sb[j][0] = wh
            for j in range(NP):
                wz = singles.tile([128, 2, H // 2], F8, tag=f"wz{j}_1")
                nc.sync.dma_start(
                    out=wz,
                    in_=wz_d.ap()[256 * j:256 * (j + 1), H // 2:H].rearrange(
                        "(two p) h -> p two h", two=2
                    ),
                )
                wz_sb[j][1] = wz
                wh = singles.tile([128, 2, H // 2], F8, tag=f"wh{j}_1")
                nc.sync.dma_start(
                    out=wh,
                    in_=wh_d.ap()[256 * j:256 * (j + 1), H // 2:H].rearrange(
                        "(two p) h -> p two h", two=2
                    ),
                )
                wh_sb[j][1] = wh

            h_prev: list = [None] * NM

            # Global software pipeline over steps s = (unit, m).  Stages are
            # issued with explicit delays so every in-order engine queue only
            # ever sees instructions whose inputs are at least one step old:
            #   step s:  matmuls(s), a-act(s), z(s)=1-a on GpSimd,
            #            sp/pb-acts(s-1), g(s-1) on DVE, bb(s-1)=z*g on GpSimd,
            #            scan(s-2) + h-DMA(s-2) on DVE/SP.
            # This keeps Scalar from embedding waits on in-flight PE groups
            # (a-act is the only act coupled to a fresh GEMM) and gives bb a
            # full step of slack before the scan consumes it.
            stage1q: list = []  # (m, pp, a, z, tw, ts_sl)

            def stage1(keep):
                while len(stage1q) > keep:
                    m1, pp1, a1, z1, tw1, sl1 = stage1q.pop(0)
                    sp = work.tile([128, 1024], BF16, tag="sp")
                    nc.scalar.activation(
                        out=sp[:, :tw1], in_=pp1[:, :tw1], func=AF.Sigmoid,
                        bias=bh_sb[:, m1:m1 + 1], scale=ISCALE,
                    )
                    pb = work.tile([128, 1024], BF16, tag="pb")
                    nc.scalar.activation(
                        out=pb[:, :tw1], in_=pp1[:, :tw1], func=AF.Identity,
                        bias=bh05_sb[:, m1:m1 + 1], scale=ISCALE,
                    )
                    # g = max(sigmoid(p'), p' + 0.5)  == reference g(p')
                    g = work.tile([128, 1024], BF16, tag="g")
                    nc.vector.tensor_tensor(
                        out=g[:, :tw1], in0=sp[:, :tw1], in1=pb[:, :tw1],
                        op=OP.max,
                    )
                    # bb on DVE: keeps the g->bb->scan chain engine-local so
                    # the scheduler cannot interlock DVE with GpSimd latency
                    bb = work.tile([128, 1024], BF16, tag="bb")
                    nc.vector.tensor_tensor(
                        out=bb[:, :tw1], in0=z1[:, :tw1], in1=g[:, :tw1],
                        op=OP.mult,
                    )
                    h = hpool.tile([128, 1024], BF16, tag=f"h{m1}")
                    if h_prev[m1] is None:
                        init = 0.5
                    else:
                        pt, pw = h_prev[m1]
                        init = pt[:, pw - 1:pw]
                    nc.vector.tensor_tensor_scan(
                        out=h[:, :tw1], data0=a1[:, :tw1], data1=bb[:, :tw1],
                        initial=init, op0=OP.mult, op1=OP.add,
                    )
                    h_prev[m1] = (h, tw1)
                    nc.sync.dma_start(
                        out=hT_d.ap()[m1 * 128:(m1 + 1) * 128, sl1],
                        in_=h[:, :tw1],
                    )

            for u, (ts0, tw) in enumerate(units):
                ts_sl = slice(ts0, ts0 + tw)
                if u == 0:
                    xs = xs0
                else:
                    xs = []
                    for j in range(NP):
                        xt = xpool.tile([128, 2, 1024], F8, tag=f"xs{j}")
                        nc.sync.dma_start(
                            out=xt[:, :, :tw],
                            in_=xT_d.ap()[
                                256 * j:256 * (j + 1), ts_sl
                            ].rearrange("(two p) s -> p two s", two=2),
                        )
                        xs.append(xt)
                for m in range(NM):
                    hf, ml = divmod(m, NM // 2)
                    m_sl = slice(ml * 128, (ml + 1) * 128)
                    kp = psum.tile([128, 1024], F32, tag="kp")
                    pp = psum.tile([128, 1024], F32, tag="pp")
                    for dst, w_sb in ((kp, wz_sb), (pp, wh_sb)):
                        for c0 in range(0, tw, 512):
                            cw = min(512, tw - c0)
                            for j in range(NP):
                                nc.tensor.matmul(
                                    dst[:, c0:c0 + cw],
                                    lhsT=w_sb[j][hf][:, :, m_sl],
                                    rhs=xs[j][:, :, c0:c0 + cw],
                                    start=(j == 0),
                                    stop=(j == NP - 1),
                                    perf_mode=DR,
                                )
                    # a = sigmoid(-(k/S + bz)) = 1 - z
                    a = work.tile([128, 1024], BF16, tag="a")
                    nc.scalar.activation(
                        out=a[:, :tw], in_=kp[:, :tw], func=AF.Sigmoid,
                        bias=nbz_sb[:, m:m + 1], scale=-ISCALE,
                    )
                    # z = 1 - a on GpSimd (off the Scalar/DVE critical paths)
                    z = work.tile([128, 1024], BF16, tag="z")
                    nc.gpsimd.tensor_scalar(
                        out=z[:, :tw], in0=a[:, :tw], scalar1=-1.0,
                        scalar2=1.0, op0=OP.mult, op1=OP.add,
                    )
                    stage1q.append((m, pp, a, z, tw, ts_sl))
                    stage1(1)

            stage1(0)

    nc.compile()
    return nc


def _prep_inputs(x, Wz, bz, Wh, bh):
    import ml_dtypes

    E4 = ml_dtypes.float8_e4m3
    x = np.asarray(x, np.float32)
    xq = (x * SX).astype(E4)
    wz8 = np.ascontiguousarray((np.asarray(Wz, np.float32).T * SW).astype(E4))
    wh8 = np.ascontiguousarray((np.asarray(Wh, np.float32).T * SW).astype(E4))
    nbz = np.ascontiguousarray(-np.asarray(bz, np.float32))
    bh32 = np.ascontiguousarray(np.asarray(bh, np.float32))
    bh05 = np.ascontiguousarray(bh32 + np.float32(0.5))
    return [
        {
            "xT8": np.ascontiguousarray(xq[b].T),
            "wz8": wz8,
            "wh8": wh8,
            "nbz": nbz,
            "bh": bh32,
            "bh05": bh05,
        }
        for b in range(x.shape[0])
    ]


def kernel(x, Wz, bz, Wh, bh):
    if "nc" not in _cache:
        _cache["nc"] = build_nc()
    nc = _cache["nc"]
    in_maps = _prep_inputs(x, Wz, bz, Wh, bh)
    res = run_bass_kernel_spmd(nc, in_maps, list(range(N_CORES)))
    out = np.empty((B, S, H), np.float32)
    for b in range(N_CORES):
        out[b] = res.results[b]["hT"].astype(np.float32).T
    return out
